# revision 33
# baseline (speedup 1.0000x reference)
"""Butterfly (2x2 block-diagonal) matrix multiply on 8 Trainium2 NeuronCores.

reference:  y[..., 2k]   = t00[k]*x[..., 2k] + t01[k]*x[..., 2k+1]
            y[..., 2k+1] = t10[k]*x[..., 2k] + t11[k]*x[..., 2k+1]

Strategy: memory-bound elementwise op. Host pre-transposes x to
feature-major and deinterleaves even/odd features so that on device the
twiddle coefficients are per-partition scalars (fp32 tensor_scalar runs
2x on DVE; the fused scalar_tensor_tensor does (in0*scalar)+in1 in one
op). The 2048 feature-pairs are sharded 256 per core; every DMA is fully
contiguous. Per core: 16 MiB in + 16 MiB out, compute far under the DMA
roofline -> kernel is HBM-bandwidth-bound as intended.
"""

import numpy as np

import concourse.bass as bass
import concourse.mybir as mybir
import concourse.tile as tile
from concourse.bass_utils import run_bass_kernel_spmd
from concourse.vector_clock import ScopedClock

F32 = mybir.dt.float32

N_CORES = 8
N_PAIRS = 2048          # feature pairs total (n/2)
PAIRS_PER_CORE = N_PAIRS // N_CORES     # 256
ROWS = 8192             # flattened batch (4*2048)
FD = 2048             # free-dim subtile size
P = 128                 # SBUF partitions

# ---------------------------------------------------------------------------
# Workaround: this neuronxcc/walrus build rejects instructions carrying more
# than one sync-wait ("Too many sync wait commands", CoreV3GenImpl
# setupSyncWait). Tile attaches multiple sem-waits to single instructions
# (notably the kernel-tail drain). Splitting a multi-wait instruction into
# preceding single-wait NoOps on the same engine is semantically identical
# (engine program order).
# ---------------------------------------------------------------------------
_MAX_WAITS = 1


def _split_waits(nc, inst, add):
    si = inst.sync_info
    if si is None or not si.on_wait or len(si.on_wait) <= _MAX_WAITS:
        return
    waits = list(si.on_wait)
    for w in waits[:-_MAX_WAITS]:
        n = mybir.InstNoOp(name=nc.get_next_instruction_name(), ins=[], outs=[])
        n.engine = inst.engine
        n.sync_info = mybir.SyncInfo(on_wait=[w], on_update=[])
        add(n)
    inst.sync_info = mybir.SyncInfo(
        on_wait=waits[-_MAX_WAITS:], on_update=list(si.on_update)
    )


_orig_add_instruction = tile.TileContext._add_instruction


def _patched_add_instruction(self, inst):
    _split_waits(self.nc, inst, lambda n: _orig_add_instruction(self, n))
    _orig_add_instruction(self, inst)


def _attach_global_waits(nc, engine, tick_clock, wait_clock):
    """Emit drain(s) on `engine` that wait for the final tick of every
    processor in the tile global clock, one sync-wait per instruction."""
    drain_inst = engine.drain()
    wait_clock.add_sem_waits(
        drain_inst.ins, ScopedClock({None: tick_clock.global_clock})
    )
    mi = drain_inst.ins
    si = mi.sync_info
    if si is not None and si.on_wait and len(si.on_wait) > _MAX_WAITS:
        waits = list(si.on_wait)
        mi.sync_info = mybir.SyncInfo(on_wait=waits[:_MAX_WAITS], on_update=[])
        for w in waits[_MAX_WAITS:]:
            d2 = engine.drain()
            d2.ins.sync_info = mybir.SyncInfo(on_wait=[w], on_update=[])


def _patched_drain_and_barrier(self, tick_clock, wait_clock):
    # Replace the stock tail (drain + all-engine EVSEM barrier + Pool-side
    # sem clears + barrier, ~9-17us) with: SP waits for the global clock
    # (guarantees all DMAs landed before the NEFF completes) and Pool waits
    # for the same clock before clearing sems (keeps re-execution safe).
    # No barrier butterfly needed.
    nc = self.nc
    _attach_global_waits(nc, nc.sync, tick_clock, wait_clock)
    _attach_global_waits(nc, nc.gpsimd, tick_clock, wait_clock)
    assert self.sems is not None
    popped = nc._tile_sem_poison_stack.pop()
    assert popped is self._sem_poison
    nc.clear_and_free_semaphores(list(self.sems.allocated().values()))


tile.TileContext._add_instruction = _patched_add_instruction
tile.TileContext._drain_and_barrier = _patched_drain_and_barrier

# ---------------------------------------------------------------------------


def _build_program():
    nc = bass.Bass()
    xe = nc.declare_dram_parameter("xe", [PAIRS_PER_CORE, ROWS], F32, isOutput=False)
    xo = nc.declare_dram_parameter("xo", [PAIRS_PER_CORE, ROWS], F32, isOutput=False)
    tw = nc.declare_dram_parameter("tw", [PAIRS_PER_CORE, 4], F32, isOutput=False)
    ye = nc.declare_dram_parameter("ye", [PAIRS_PER_CORE, ROWS], F32, isOutput=True)
    yo = nc.declare_dram_parameter("yo", [PAIRS_PER_CORE, ROWS], F32, isOutput=True)

    n_chunks = PAIRS_PER_CORE // P      # partition chunks (2)
    n_sub = ROWS // FD                  # free-dim subtiles (4)

    with tile.TileContext(nc) as tc:
        with (
            tc.tile_pool(name="coef", bufs=n_chunks) as cpool,
            tc.tile_pool(name="io", bufs=3) as pool,
        ):
            # prefetch iteration-0 inputs FIRST so SP's first dispatch is the
            # big load; the tiny coefficient loads ride the idle qACT queue
            pre_xet = pool.tile([P, FD], F32, tag="xet")
            nc.sync.dma_start(out=pre_xet[:], in_=xe[0:P, 0:FD])
            pre_xot = pool.tile([P, FD], F32, tag="xot")
            # second first-load on the qACT generator: descriptor generation
            # is ~3us per 128-partition DMA per generator, so issuing the
            # first two loads on different generators overlaps their ramps
            # (safe on qACT: precedes every output in ACT program order)
            nc.scalar.dma_start(out=pre_xot[:], in_=xo[0:P, 0:FD])
            cts = []
            for c in range(n_chunks):
                ctc = cpool.tile([P, 4], F32, tag="ct")
                nc.scalar.dma_start(out=ctc[:], in_=tw[c * P : (c + 1) * P, :])
                cts.append(ctc)
            for c in range(n_chunks):
                ps = slice(c * P, (c + 1) * P)
                ct = cts[c]
                t00, t01 = ct[:, 0:1], ct[:, 1:2]
                t10, t11 = ct[:, 2:3], ct[:, 3:4]
                # split the very last subtile in half so the closing
                # input->compute->output dependency chain is shorter
                spans = [(j * FD, (j + 1) * FD) for j in range(n_sub)]
                if c == n_chunks - 1:
                    lo, hi = spans.pop()
                    q = (hi - lo) // 4
                    spans += [(lo + i * q, lo + (i + 1) * q) for i in range(4)]
                # pair up consecutive equal-size spans: one 2x-wide load
                # dispatch feeds two compute subtiles (bigger descriptors
                # run closer to per-engine peak), stores stay at FD
                loads = {}
                i = 0
                while i < len(spans):
                    lo, hi = spans[i]
                    if (
                        i + 1 < len(spans)
                        and spans[i + 1][1] - spans[i + 1][0] == hi - lo
                        and not (c == 0 and lo == 0)
                    ):
                        big = hi - lo + spans[i + 1][1] - spans[i + 1][0]
                        xet2 = pool.tile([P, big], F32, tag="xet")
                        nc.sync.dma_start(
                            out=xet2[:], in_=xe[ps, lo : lo + big]
                        )
                        xot2 = pool.tile([P, big], F32, tag="xot")
                        nc.sync.dma_start(
                            out=xot2[:], in_=xo[ps, lo : lo + big]
                        )
                        w = hi - lo
                        loads[spans[i]] = (xet2[:, 0:w], xot2[:, 0:w])
                        loads[spans[i + 1]] = (
                            xet2[:, w : 2 * w],
                            xot2[:, w : 2 * w],
                        )
                        i += 2
                    else:
                        i += 1
                for lo, hi in spans:
                    fs = slice(lo, hi)
                    fd = hi - lo
                    if c == 0 and lo == 0:
                        xet, xot = pre_xet[:], pre_xot[:]
                    elif (lo, hi) in loads:
                        xet, xot = loads[(lo, hi)]
                    else:
                        xet_t = pool.tile([P, fd], F32, tag="xet")
                        nc.sync.dma_start(out=xet_t[:], in_=xe[ps, fs])
                        xot_t = pool.tile([P, fd], F32, tag="xot")
                        nc.sync.dma_start(out=xot_t[:], in_=xo[ps, fs])
                        xet, xot = xet_t[:], xot_t[:]

                    # all compute on DVE (tensor_scalar runs 2x for fp32;
                    # ACT only dispatches output DMAs so its sem stalls
                    # never delay compute)
                    yet = pool.tile([P, fd], F32, tag="yet")
                    # yet = t01*xo
                    nc.vector.tensor_scalar_mul(yet[:], xot[:], t01)
                    # yet = (xe*t00) + yet  (fused mult-add)
                    nc.vector.scalar_tensor_tensor(
                        yet[:], xet[:], t00, yet[:],
                        op0=mybir.AluOpType.mult, op1=mybir.AluOpType.add,
                    )
                    # outputs go out on the Activation HWDGE queue so they
                    # never head-of-line-block ready input loads on qSP
                    nc.scalar.dma_start(out=ye[ps, fs], in_=yet[:])

                    yot = pool.tile([P, fd], F32, tag="yot")
                    # yot = t10*xe
                    nc.vector.tensor_scalar_mul(yot[:], xet[:], t10)
                    # yot = (xo*t11) + yot
                    nc.vector.scalar_tensor_tensor(
                        yot[:], xot[:], t11, yot[:],
                        op0=mybir.AluOpType.mult, op1=mybir.AluOpType.add,
                    )
                    nc.scalar.dma_start(out=yo[ps, fs], in_=yot[:])
    return nc


_nc_cache = None


def _get_program():
    global _nc_cache
    if _nc_cache is None:
        _nc_cache = _build_program()
    return _nc_cache


def _run(in_maps, **kw):
    nc = _get_program()
    last_exc = None
    for attempt in range(3):
        try:
            return run_bass_kernel_spmd(nc, in_maps, list(range(N_CORES)), **kw)
        except Exception as e:  # transient NRT_EXEC_UNIT_UNRECOVERABLE seen once
            last_exc = e
            if attempt < 2:
                import time

                time.sleep(5.0)
    raise last_exc


def kernel(x: np.ndarray, twiddle: np.ndarray, _run_kw=None) -> np.ndarray:
    x = np.asarray(x, dtype=np.float32)
    twiddle = np.asarray(twiddle, dtype=np.float32)
    orig_shape = x.shape
    n = orig_shape[-1]
    assert n == 2 * N_PAIRS, (orig_shape, N_PAIRS)

    xr = np.ascontiguousarray(x).reshape(-1, N_PAIRS, 2)   # [ROWS, n/2, 2]
    assert xr.shape[0] == ROWS, xr.shape
    # feature-major, even/odd split: [N_PAIRS, ROWS]
    xe_all = np.ascontiguousarray(xr[:, :, 0].T)
    xo_all = np.ascontiguousarray(xr[:, :, 1].T)
    tw_all = np.ascontiguousarray(twiddle.reshape(N_PAIRS, 4))

    in_maps = []
    for i in range(N_CORES):
        ks = slice(i * PAIRS_PER_CORE, (i + 1) * PAIRS_PER_CORE)
        in_maps.append({"xe": xe_all[ks], "xo": xo_all[ks], "tw": tw_all[ks]})

    res = _run(in_maps, **(_run_kw or {}))

    out = np.empty((ROWS, N_PAIRS, 2), dtype=np.float32)
    for i in range(N_CORES):
        ks = slice(i * PAIRS_PER_CORE, (i + 1) * PAIRS_PER_CORE)
        out[:, ks, 0] = res.results[i]["ye"].T
        out[:, ks, 1] = res.results[i]["yo"].T
    result = out.reshape(orig_shape)

    if _run_kw:
        # expose profiling info to the caller's harness
        kernel.last_results = res
    return result


# revision 35
# speedup vs baseline: 1.1678x; 1.1678x over previous
"""Butterfly (2x2 block-diagonal) matrix multiply on 8 Trainium2 NeuronCores.

reference:  y[..., 2k]   = t00[k]*x[..., 2k] + t01[k]*x[..., 2k+1]
            y[..., 2k+1] = t10[k]*x[..., 2k] + t11[k]*x[..., 2k+1]

Strategy: memory-bound elementwise op. Host pre-transposes x to
feature-major and deinterleaves even/odd features so that on device the
twiddle coefficients are per-partition scalars (fp32 tensor_scalar runs
2x on DVE; the fused scalar_tensor_tensor does (in0*scalar)+in1 in one
op). The 2048 feature-pairs are sharded 256 per core; every DMA is fully
contiguous. Per core: 16 MiB in + 16 MiB out, compute far under the DMA
roofline -> kernel is HBM-bandwidth-bound as intended.
"""

import numpy as np

import concourse.bass as bass
import concourse.mybir as mybir
import concourse.tile as tile
from concourse.bass_utils import run_bass_kernel_spmd
from concourse.vector_clock import ScopedClock

F32 = mybir.dt.float32

N_CORES = 8
N_PAIRS = 2048          # feature pairs total (n/2)
PAIRS_PER_CORE = N_PAIRS // N_CORES     # 256
ROWS = 8192             # flattened batch (4*2048)
FD = 2048             # free-dim subtile size
P = 128                 # SBUF partitions

# ---------------------------------------------------------------------------
# Workaround: this neuronxcc/walrus build rejects instructions carrying more
# than one sync-wait ("Too many sync wait commands", CoreV3GenImpl
# setupSyncWait). Tile attaches multiple sem-waits to single instructions
# (notably the kernel-tail drain). Splitting a multi-wait instruction into
# preceding single-wait NoOps on the same engine is semantically identical
# (engine program order).
# ---------------------------------------------------------------------------
_MAX_WAITS = 1


def _split_waits(nc, inst, add):
    si = inst.sync_info
    if si is None or not si.on_wait or len(si.on_wait) <= _MAX_WAITS:
        return
    waits = list(si.on_wait)
    for w in waits[:-_MAX_WAITS]:
        n = mybir.InstNoOp(name=nc.get_next_instruction_name(), ins=[], outs=[])
        n.engine = inst.engine
        n.sync_info = mybir.SyncInfo(on_wait=[w], on_update=[])
        add(n)
    inst.sync_info = mybir.SyncInfo(
        on_wait=waits[-_MAX_WAITS:], on_update=list(si.on_update)
    )


_orig_add_instruction = tile.TileContext._add_instruction


def _patched_add_instruction(self, inst):
    _split_waits(self.nc, inst, lambda n: _orig_add_instruction(self, n))
    _orig_add_instruction(self, inst)


def _attach_global_waits(nc, engine, tick_clock, wait_clock):
    """Emit drain(s) on `engine` that wait for the final tick of every
    processor in the tile global clock, one sync-wait per instruction."""
    drain_inst = engine.drain()
    wait_clock.add_sem_waits(
        drain_inst.ins, ScopedClock({None: tick_clock.global_clock})
    )
    mi = drain_inst.ins
    si = mi.sync_info
    if si is not None and si.on_wait and len(si.on_wait) > _MAX_WAITS:
        waits = list(si.on_wait)
        mi.sync_info = mybir.SyncInfo(on_wait=waits[:_MAX_WAITS], on_update=[])
        for w in waits[_MAX_WAITS:]:
            d2 = engine.drain()
            d2.ins.sync_info = mybir.SyncInfo(on_wait=[w], on_update=[])


def _patched_drain_and_barrier(self, tick_clock, wait_clock):
    # Replace the stock tail (drain + all-engine EVSEM barrier + Pool-side
    # sem clears + barrier, ~9-17us) with: SP waits for the global clock
    # (guarantees all DMAs landed before the NEFF completes) and Pool waits
    # for the same clock before clearing sems (keeps re-execution safe).
    # No barrier butterfly needed.
    nc = self.nc
    _attach_global_waits(nc, nc.sync, tick_clock, wait_clock)
    _attach_global_waits(nc, nc.gpsimd, tick_clock, wait_clock)
    assert self.sems is not None
    popped = nc._tile_sem_poison_stack.pop()
    assert popped is self._sem_poison
    nc.clear_and_free_semaphores(list(self.sems.allocated().values()))


tile.TileContext._add_instruction = _patched_add_instruction
tile.TileContext._drain_and_barrier = _patched_drain_and_barrier

# ---------------------------------------------------------------------------


def _build_program():
    nc = bass.Bass()
    xe = nc.declare_dram_parameter("xe", [PAIRS_PER_CORE, ROWS], F32, isOutput=False)
    xo = nc.declare_dram_parameter("xo", [PAIRS_PER_CORE, ROWS], F32, isOutput=False)
    tw = nc.declare_dram_parameter("tw", [PAIRS_PER_CORE, 4], F32, isOutput=False)
    ye = nc.declare_dram_parameter("ye", [PAIRS_PER_CORE, ROWS], F32, isOutput=True)
    yo = nc.declare_dram_parameter("yo", [PAIRS_PER_CORE, ROWS], F32, isOutput=True)

    n_chunks = PAIRS_PER_CORE // P      # partition chunks (2)
    n_sub = ROWS // FD                  # free-dim subtiles (4)

    with tile.TileContext(nc) as tc:
        with (
            tc.tile_pool(name="coef", bufs=n_chunks) as cpool,
            tc.tile_pool(name="io", bufs=4) as pool,
        ):
            # prefetch iteration-0 inputs FIRST so SP's first dispatch is the
            # big load; the tiny coefficient loads ride the idle qACT queue
            pre_xet = pool.tile([P, FD], F32, tag="xet")
            nc.sync.dma_start(out=pre_xet[:], in_=xe[0:P, 0:FD])
            pre_xot = pool.tile([P, FD], F32, tag="xot")
            # second first-load on the qACT generator: descriptor generation
            # is ~3us per 128-partition DMA per generator, so issuing the
            # first two loads on different generators overlaps their ramps
            # (safe on qACT: precedes every output in ACT program order)
            nc.scalar.dma_start(out=pre_xot[:], in_=xo[0:P, 0:FD])
            cts = []
            for c in range(n_chunks):
                ctc = cpool.tile([P, 4], F32, tag="ct")
                nc.scalar.dma_start(out=ctc[:], in_=tw[c * P : (c + 1) * P, :])
                cts.append(ctc)
            for c in range(n_chunks):
                ps = slice(c * P, (c + 1) * P)
                ct = cts[c]
                t00, t01 = ct[:, 0:1], ct[:, 1:2]
                t10, t11 = ct[:, 2:3], ct[:, 3:4]
                # split the very last subtile in half so the closing
                # input->compute->output dependency chain is shorter
                spans = [(j * FD, (j + 1) * FD) for j in range(n_sub)]
                if c == n_chunks - 1:
                    lo, hi = spans.pop()
                    q = (hi - lo) // 4
                    spans += [(lo + i * q, lo + (i + 1) * q) for i in range(4)]
                for lo, hi in spans:
                    fs = slice(lo, hi)
                    fd = hi - lo
                    if c == 0 and lo == 0:
                        xet, xot = pre_xet, pre_xot
                    else:
                        xet = pool.tile([P, fd], F32, tag="xet")
                        nc.sync.dma_start(out=xet[:], in_=xe[ps, fs])
                        xot = pool.tile([P, fd], F32, tag="xot")
                        nc.sync.dma_start(out=xot[:], in_=xo[ps, fs])

                    # all compute on DVE (tensor_scalar runs 2x for fp32;
                    # ACT only dispatches output DMAs so its sem stalls
                    # never delay compute)
                    yet = pool.tile([P, fd], F32, tag="yet")
                    # yet = t01*xo
                    nc.vector.tensor_scalar_mul(yet[:], xot[:], t01)
                    # yet = (xe*t00) + yet  (fused mult-add)
                    nc.vector.scalar_tensor_tensor(
                        yet[:], xet[:], t00, yet[:],
                        op0=mybir.AluOpType.mult, op1=mybir.AluOpType.add,
                    )
                    # outputs go out on the Activation HWDGE queue so they
                    # never head-of-line-block ready input loads on qSP
                    nc.scalar.dma_start(out=ye[ps, fs], in_=yet[:])

                    yot = pool.tile([P, fd], F32, tag="yot")
                    # yot = t10*xe
                    nc.vector.tensor_scalar_mul(yot[:], xet[:], t10)
                    # yot = (xo*t11) + yot
                    nc.vector.scalar_tensor_tensor(
                        yot[:], xot[:], t11, yot[:],
                        op0=mybir.AluOpType.mult, op1=mybir.AluOpType.add,
                    )
                    nc.scalar.dma_start(out=yo[ps, fs], in_=yot[:])
    return nc


_nc_cache = None


def _get_program():
    global _nc_cache
    if _nc_cache is None:
        _nc_cache = _build_program()
    return _nc_cache


def _run(in_maps, **kw):
    nc = _get_program()
    last_exc = None
    for attempt in range(3):
        try:
            return run_bass_kernel_spmd(nc, in_maps, list(range(N_CORES)), **kw)
        except Exception as e:  # transient NRT_EXEC_UNIT_UNRECOVERABLE seen once
            last_exc = e
            if attempt < 2:
                import time

                time.sleep(5.0)
    raise last_exc


def kernel(x: np.ndarray, twiddle: np.ndarray, _run_kw=None) -> np.ndarray:
    x = np.asarray(x, dtype=np.float32)
    twiddle = np.asarray(twiddle, dtype=np.float32)
    orig_shape = x.shape
    n = orig_shape[-1]
    assert n == 2 * N_PAIRS, (orig_shape, N_PAIRS)

    xr = np.ascontiguousarray(x).reshape(-1, N_PAIRS, 2)   # [ROWS, n/2, 2]
    assert xr.shape[0] == ROWS, xr.shape
    # feature-major, even/odd split: [N_PAIRS, ROWS]
    xe_all = np.ascontiguousarray(xr[:, :, 0].T)
    xo_all = np.ascontiguousarray(xr[:, :, 1].T)
    tw_all = np.ascontiguousarray(twiddle.reshape(N_PAIRS, 4))

    in_maps = []
    for i in range(N_CORES):
        ks = slice(i * PAIRS_PER_CORE, (i + 1) * PAIRS_PER_CORE)
        in_maps.append({"xe": xe_all[ks], "xo": xo_all[ks], "tw": tw_all[ks]})

    res = _run(in_maps, **(_run_kw or {}))

    out = np.empty((ROWS, N_PAIRS, 2), dtype=np.float32)
    for i in range(N_CORES):
        ks = slice(i * PAIRS_PER_CORE, (i + 1) * PAIRS_PER_CORE)
        out[:, ks, 0] = res.results[i]["ye"].T
        out[:, ks, 1] = res.results[i]["yo"].T
    result = out.reshape(orig_shape)

    if _run_kw:
        # expose profiling info to the caller's harness
        kernel.last_results = res
    return result
